# revision 4
# baseline (speedup 1.0000x reference)
"""Causal self-attention (BS=4, SL=2048, NE=1024, NH=16) on 8 trn2 NeuronCores.

Sharding (uniform SPMD program on all 8 cores):
  core c -> batch b = c//2, head-group g = c%2 (8 of 16 heads, 512 feats).
  Each core: QKV proj for its heads (full 2048 rows of its batch),
  causal attention for its 8 heads, then pairwise AllGather of y
  (cores 2b/2b+1), and out-proj for a 512-column half of the output.
  Host reassembles [4, 2048, 1024] from the 8 [2048, 512] halves.

Matmul operands in fp16 (full PE rate, fp32 PSUM accumulate).
Attention computed in S^T = K @ Q^T layout so that:
  - PV needs no transposes: Y^T[65,q] += [V|1]^T @ expS^T (row 64 = denom)
  - softmax normalization via gpsimd partition_broadcast of 1/denom
"""

import sys

if "/opt/trn_rl_repo" not in sys.path:
    sys.path.insert(0, "/opt/trn_rl_repo")

import numpy as np

import concourse.bass as bass
import concourse.mybir as mybir
import concourse.tile as tile
from concourse import bacc
from concourse.bass_utils import run_bass_kernel_spmd

F32 = mybir.dt.float32
F16 = mybir.dt.float16

# problem dims (hardcoded per spec)
BS, SL, NE, NH = 4, 2048, 1024, 16
HD = 64
N_CORES = 8


def build_nc(sl=SL, ne=NE, nh=NH):
    """Build the per-core Bass program. All 8 cores run this identically."""
    H = nh // 2          # local heads per core
    F = H * HD           # local feats (q/k/v width per core)
    FG = F // 128        # feat groups of 128 (2 heads each)
    CH = ne // 128       # contraction chunks for the projections
    PANEL = 512          # q-panel width
    NP = sl // PANEL     # number of q panels
    NKB = sl // 128      # number of 128-row k blocks
    OUTW = ne // 2       # out-proj columns computed per core
    VW = H * 65          # V' width (65-stride per head: 64 V cols + ones)

    nc = bacc.Bacc("TRN2", target_bir_lowering=False, num_devices=N_CORES)

    x = nc.dram_tensor("x", [sl, ne], F32, kind="ExternalInput")
    wq = nc.dram_tensor("wq", [ne, F], F32, kind="ExternalInput")
    wk = nc.dram_tensor("wk", [ne, F], F32, kind="ExternalInput")
    wv = nc.dram_tensor("wv", [ne, F], F32, kind="ExternalInput")
    bq = nc.dram_tensor("bq", [F], F32, kind="ExternalInput")
    bk = nc.dram_tensor("bk", [F], F32, kind="ExternalInput")
    bv = nc.dram_tensor("bv", [F], F32, kind="ExternalInput")
    wo = nc.dram_tensor("wo", [ne, OUTW], F32, kind="ExternalInput")
    bo = nc.dram_tensor("bo", [OUTW], F32, kind="ExternalInput")
    out = nc.dram_tensor("out", [sl, OUTW], F32, kind="ExternalOutput")

    ident_dram = nc.inline_tensor(np.eye(128, dtype=np.float16), name="ident_c")
    tri_dram = nc.inline_tensor(
        np.triu(np.ones((128, 128), dtype=np.float16)), name="tri_c")

    with tile.TileContext(nc) as tc:
        with (
            tc.tile_pool(name="consts", bufs=1) as consts,
            tc.tile_pool(name="xload", bufs=3) as xload,
            tc.tile_pool(name="xt", bufs=2) as xtp,
            tc.tile_pool(name="qt", bufs=2) as qtp,
            tc.tile_pool(name="persist", bufs=1) as persist,
            tc.tile_pool(name="es", bufs=4) as esp,
            tc.tile_pool(name="ny", bufs=2) as nyp,
            tc.tile_pool(name="misc", bufs=2) as misc,
            tc.tile_pool(name="psum", bufs=1, space="PSUM") as psp,
            tc.tile_pool(name="dram", bufs=1, space="DRAM") as dram,
        ):
            # ---- constants ----
            ident = consts.tile([128, 128], F16)
            nc.sync.dma_start(out=ident, in_=ident_dram[:])
            tri = consts.tile([128, 128], F16)
            nc.sync.dma_start(out=tri, in_=tri_dram[:])
            ones = consts.tile([1, 128], F16)
            nc.vector.memset(ones, 1.0)
            bqt = consts.tile([128, FG], F32)
            nc.sync.dma_start(out=bqt, in_=bq.rearrange("(g p) -> p g", p=128))
            bkt = consts.tile([128, FG], F32)
            nc.sync.dma_start(out=bkt, in_=bk.rearrange("(g p) -> p g", p=128))
            bvt = consts.tile([1, F], F16)
            nc.gpsimd.dma_start(out=bvt, in_=bv.rearrange("(a n) -> a n", a=1))
            bot = consts.tile([1, OUTW], F16)
            nc.gpsimd.dma_start(out=bot, in_=bo.rearrange("(a n) -> a n", a=1))

            # ---- resident weights (cast fp32 -> fp16 in the DMA) ----
            WQ = [persist.tile([128, F], F16, tag=f"wq{c}", name=f"WQ{c}")
                  for c in range(CH)]
            WK = [persist.tile([128, F], F16, tag=f"wk{c}", name=f"WK{c}")
                  for c in range(CH)]
            WV = [persist.tile([128, F], F16, tag=f"wv{c}", name=f"WV{c}")
                  for c in range(CH)]
            WO = [persist.tile([128, OUTW], F16, tag=f"wo{c}", name=f"WO{c}")
                  for c in range(CH)]
            for c in range(CH):
                sl_c = slice(c * 128, (c + 1) * 128)
                nc.gpsimd.dma_start(out=WQ[c], in_=wq[sl_c, :])
                nc.gpsimd.dma_start(out=WK[c], in_=wk[sl_c, :])
                nc.gpsimd.dma_start(out=WV[c], in_=wv[sl_c, :])
                nc.gpsimd.dma_start(out=WO[c], in_=wo[sl_c, :])

            # ---- persistent attention operands ----
            KT = [persist.tile([128, sl], F16, tag=f"kt{f}", name=f"KT{f}")
                  for f in range(FG)]
            VP = [persist.tile([128, VW], F16, tag=f"vp{k}", name=f"VP{k}")
                  for k in range(NKB)]

            y_local = dram.tile([F, sl], F16)
            y_all = dram.tile([2, F, sl], F16)

            for p in range(NP):
                # ---------- projection for panel p ----------
                xT = [xtp.tile([128, PANEL], F16, tag=f"xt{c}", name=f"xT{c}")
                      for c in range(CH)]
                for sub in range(4):
                    rows = p * PANEL + sub * 128
                    x_t = xload.tile([128, ne], F16, name="x_t")
                    nc.gpsimd.dma_start(out=x_t, in_=x[rows:rows + 128, :])
                    for c in range(CH):
                        ps_t = psp.tile([128, 128], F16, tag="tr", name="ps_t")
                        nc.tensor.transpose(
                            ps_t, x_t[:, c * 128:(c + 1) * 128], ident)
                        nc.vector.tensor_copy(
                            xT[c][:, sub * 128:(sub + 1) * 128], ps_t)

                # Q^T and K^T for this panel: [F, PANEL] in FG tiles
                QT = [qtp.tile([128, PANEL], F16, tag=f"qt{f}", name=f"QT{f}")
                      for f in range(FG)]
                for f in range(FG):
                    for wtiles, dst, bias in (
                        (WQ, QT[f], bqt), (WK, KT[f], bkt)):
                        ps_a = psp.tile([128, PANEL], F32, tag="acc", name="ps_a")
                        for c in range(CH):
                            nc.tensor.matmul(
                                ps_a, wtiles[c][:, f * 128:(f + 1) * 128],
                                xT[c], start=(c == 0), stop=(c == CH - 1))
                        if dst is QT[f]:
                            nc.vector.tensor_scalar_add(
                                dst, ps_a, bias[:, f:f + 1])
                        else:
                            nc.vector.tensor_scalar_add(
                                dst[:, p * PANEL:(p + 1) * PANEL], ps_a,
                                bias[:, f:f + 1])

                # V for this panel -> V' tiles (65-stride, ones col)
                for sub in range(4):
                    kb = p * 4 + sub
                    ps_v = psp.tile([128, F], F32, tag="acc", name="ps_v")
                    for c in range(CH):
                        nc.tensor.matmul(
                            ps_v, xT[c][:, sub * 128:(sub + 1) * 128],
                            WV[c], start=(c == 0), stop=False)
                    # + bias row (broadcast via K=1 matmul)
                    nc.tensor.matmul(ps_v, ones, bvt, start=False, stop=True)
                    vp3 = VP[kb].rearrange("p (h e) -> p h e", e=65)
                    nc.vector.memset(vp3[:, :, 64:65], 1.0)
                    nc.vector.tensor_copy(
                        vp3[:, :, 0:64],
                        ps_v.rearrange("p (h d) -> p h d", d=64))

                # ---------- attention for panel p ----------
                nY = [nyp.tile([64, PANEL], F16, tag=f"ny{h}", name=f"nY{h}")
                      for h in range(H)]
                for h in range(H):
                    f, row = h // 2, (h % 2) * 64
                    ps_y = psp.tile([65, PANEL], F32, tag="y", name="ps_y")
                    nkb_p = 4 * p + 4
                    for kb in range(nkb_p):
                        d = max(0, (kb - 4 * p) * 128)
                        n = PANEL - d
                        ps_s = psp.tile([128, PANEL], F32, tag="s", name="ps_s")
                        nc.tensor.matmul(
                            ps_s[:, 0:n],
                            KT[f][row:row + 64, kb * 128:(kb + 1) * 128],
                            QT[f][row:row + 64, d:PANEL])
                        es = esp.tile([128, PANEL], F16, tag="es", name="es")
                        nc.scalar.activation(
                            es[:, 0:n], ps_s[:, 0:n],
                            mybir.ActivationFunctionType.Exp)
                        if kb >= 4 * p:
                            nc.vector.tensor_mul(
                                es[:, 0:128], es[:, 0:128], tri)
                        nc.tensor.matmul(
                            ps_y[:, d:PANEL],
                            VP[kb][:, h * 65:h * 65 + 65],
                            es[:, 0:n],
                            start=(kb == 0), stop=(kb == nkb_p - 1))
                    # normalize: 1/denom broadcast over the 64 hd rows
                    recip = misc.tile([1, PANEL], F32, tag="recip", name="recip")
                    nc.vector.reciprocal(recip, ps_y[64:65, :])
                    bc = misc.tile([64, PANEL], F32, tag="bc", name="bc")
                    nc.gpsimd.partition_broadcast(bc, recip)
                    nc.scalar.copy(nY[h], ps_y[0:64, :])
                    nc.vector.tensor_mul(nY[h], nY[h], bc)
                for h in range(H):
                    nc.sync.dma_start(
                        out=y_local[h * 64:(h + 1) * 64,
                                    p * PANEL:(p + 1) * PANEL],
                        in_=nY[h])

            # ---------- exchange y between the two cores of this batch ----
            nc.gpsimd.collective_compute(
                "AllGather",
                mybir.AluOpType.bypass,
                replica_groups=[[0, 1], [2, 3], [4, 5], [6, 7]],
                ins=[y_local.opt()],
                outs=[y_all.opt()],
            )
            y_flat = y_all.rearrange("g f s -> (g f) s")

            # ---------- out projection (our 512-column half) ----------
            for sb in range(sl // 128):
                ps_o = psp.tile([128, OUTW], F32, tag="acc", name="ps_o")
                for c in range(CH):
                    y_t = misc.tile([128, 128], F16, tag="y_t", bufs=4,
                                    name="y_t")
                    nc.sync.dma_start(
                        out=y_t,
                        in_=y_flat[c * 128:(c + 1) * 128,
                                   sb * 128:(sb + 1) * 128])
                    nc.tensor.matmul(ps_o, y_t, WO[c],
                                     start=(c == 0), stop=False)
                nc.tensor.matmul(ps_o, ones, bot, start=False, stop=True)
                o_t = misc.tile([128, OUTW], F32, tag="o_t", bufs=3, name="o_t")
                nc.vector.tensor_copy(o_t, ps_o)
                nc.sync.dma_start(
                    out=out[sb * 128:(sb + 1) * 128, :], in_=o_t)

    nc.compile()
    return nc


def shard_inputs(x, mask, Wqkv, bqkv, Wo, bo, sl=SL, ne=NE, nh=NH):
    """Host-side sharding: returns in_maps for the 8 cores."""
    H = nh // 2
    F = H * HD
    scale = 1.0 / np.sqrt(HD)
    in_maps = []
    for c in range(N_CORES):
        b, g = c // 2, c % 2
        qc = slice(g * F, (g + 1) * F)
        kc = slice(ne + g * F, ne + (g + 1) * F)
        vc = slice(2 * ne + g * F, 2 * ne + (g + 1) * F)
        oc = slice(g * (ne // 2), (g + 1) * (ne // 2))
        in_maps.append({
            "x": np.ascontiguousarray(x[b]),
            "wq": np.ascontiguousarray(Wqkv[:, qc]) * scale,
            "wk": np.ascontiguousarray(Wqkv[:, kc]),
            "wv": np.ascontiguousarray(Wqkv[:, vc]),
            "bq": np.ascontiguousarray(bqkv[qc]) * scale,
            "bk": np.ascontiguousarray(bqkv[kc]),
            "bv": np.ascontiguousarray(bqkv[vc]),
            "wo": np.ascontiguousarray(Wo[:, oc]),
            "bo": np.ascontiguousarray(bo[oc]),
        })
    return in_maps


def unshard_output(results, sl=SL, ne=NE):
    out = np.empty((BS, sl, ne), dtype=np.float32)
    half = ne // 2
    for c in range(N_CORES):
        b, g = c // 2, c % 2
        out[b, :, g * half:(g + 1) * half] = results[c]["out"]
    return out


_NC_CACHE = {}


def kernel(x, mask, Wqkv, bqkv, Wo, bo):
    x = np.asarray(x, dtype=np.float32)
    Wqkv = np.asarray(Wqkv, dtype=np.float32)
    bqkv = np.asarray(bqkv, dtype=np.float32)
    Wo = np.asarray(Wo, dtype=np.float32)
    bo = np.asarray(bo, dtype=np.float32)
    if "nc" not in _NC_CACHE:
        _NC_CACHE["nc"] = build_nc()
    nc = _NC_CACHE["nc"]
    in_maps = shard_inputs(x, mask, Wqkv, bqkv, Wo, bo)
    res = run_bass_kernel_spmd(nc, in_maps, list(range(N_CORES)))
    return unshard_output(res.results)


# revision 5
# speedup vs baseline: 1.7645x; 1.7645x over previous
"""Causal self-attention (BS=4, SL=2048, NE=1024, NH=16) on 8 trn2 NeuronCores.

Sharding (uniform SPMD program on all 8 cores):
  core c -> batch b = c//2, head-group g = c%2 (8 of 16 heads, 512 feats).
  Each core: QKV proj for its heads (full 2048 rows of its batch),
  causal attention for its 8 heads, then pairwise AllGather of y
  (cores 2b/2b+1), and out-proj for a 512-column half of the output.
  Host reassembles [4, 2048, 1024] from the 8 [2048, 512] halves.

Matmul operands in fp16 (full PE rate, fp32 PSUM accumulate).
Attention computed in S^T = K @ Q^T layout so that:
  - PV needs no transposes: Y^T[65,q] += [V|1]^T @ expS^T (row 64 = denom)
  - softmax normalization via gpsimd partition_broadcast of 1/denom
"""

import sys

if "/opt/trn_rl_repo" not in sys.path:
    sys.path.insert(0, "/opt/trn_rl_repo")

import numpy as np

import concourse.bass as bass
import concourse.mybir as mybir
import concourse.tile as tile
from concourse import bacc
from concourse.bass_utils import run_bass_kernel_spmd

F32 = mybir.dt.float32
F16 = mybir.dt.float16

# problem dims (hardcoded per spec)
BS, SL, NE, NH = 4, 2048, 1024, 16
HD = 64
N_CORES = 8


def build_nc(sl=SL, ne=NE, nh=NH):
    """Build the per-core Bass program. All 8 cores run this identically."""
    H = nh // 2          # local heads per core
    F = H * HD           # local feats (q/k/v width per core)
    FG = F // 128        # feat groups of 128 (2 heads each)
    CH = ne // 128       # contraction chunks for the projections
    PANEL = 512          # q-panel width
    NP = sl // PANEL     # number of q panels
    NKB = sl // 128      # number of 128-row k blocks
    OUTW = ne // 2       # out-proj columns computed per core
    VW = H * 65          # V' width (65-stride per head: 64 V cols + ones)

    nc = bacc.Bacc("TRN2", target_bir_lowering=False, num_devices=N_CORES)

    x = nc.dram_tensor("x", [sl, ne], F16, kind="ExternalInput")
    wq = nc.dram_tensor("wq", [ne, F], F16, kind="ExternalInput")
    wk = nc.dram_tensor("wk", [ne, F], F16, kind="ExternalInput")
    wv = nc.dram_tensor("wv", [ne, F], F16, kind="ExternalInput")
    bq = nc.dram_tensor("bq", [F], F32, kind="ExternalInput")
    bk = nc.dram_tensor("bk", [F], F32, kind="ExternalInput")
    bv = nc.dram_tensor("bv", [F], F16, kind="ExternalInput")
    wo = nc.dram_tensor("wo", [ne, OUTW], F16, kind="ExternalInput")
    bo = nc.dram_tensor("bo", [OUTW], F16, kind="ExternalInput")
    out = nc.dram_tensor("out", [sl, OUTW], F32, kind="ExternalOutput")

    ident_dram = nc.inline_tensor(np.eye(128, dtype=np.float16), name="ident_c")
    tri_dram = nc.inline_tensor(
        np.triu(np.ones((128, 128), dtype=np.float16)), name="tri_c")

    with tile.TileContext(nc) as tc:
        with (
            tc.tile_pool(name="consts", bufs=1) as consts,
            tc.tile_pool(name="xload", bufs=3) as xload,
            tc.tile_pool(name="xt", bufs=2) as xtp,
            tc.tile_pool(name="qt", bufs=2) as qtp,
            tc.tile_pool(name="persist", bufs=1) as persist,
            tc.tile_pool(name="es", bufs=4) as esp,
            tc.tile_pool(name="ny", bufs=2) as nyp,
            tc.tile_pool(name="misc", bufs=2) as misc,
            tc.tile_pool(name="psum", bufs=1, space="PSUM") as psp,
            tc.tile_pool(name="dram", bufs=1, space="DRAM") as dram,
        ):
            # ---- constants ----
            ident = consts.tile([128, 128], F16)
            nc.sync.dma_start(out=ident, in_=ident_dram[:])
            tri = consts.tile([128, 128], F16)
            nc.sync.dma_start(out=tri, in_=tri_dram[:])
            ones = consts.tile([1, 128], F16)
            nc.vector.memset(ones, 1.0)
            bqt = consts.tile([128, FG], F32)
            nc.sync.dma_start(out=bqt, in_=bq.rearrange("(g p) -> p g", p=128))
            bkt = consts.tile([128, FG], F32)
            nc.sync.dma_start(out=bkt, in_=bk.rearrange("(g p) -> p g", p=128))
            bvt = consts.tile([1, F], F16)
            nc.sync.dma_start(out=bvt, in_=bv.rearrange("(a n) -> a n", a=1))
            bot = consts.tile([1, OUTW], F16)
            nc.sync.dma_start(out=bot, in_=bo.rearrange("(a n) -> a n", a=1))

            # ---- resident weights (cast fp32 -> fp16 in the DMA) ----
            WQ = [persist.tile([128, F], F16, tag=f"wq{c}", name=f"WQ{c}")
                  for c in range(CH)]
            WK = [persist.tile([128, F], F16, tag=f"wk{c}", name=f"WK{c}")
                  for c in range(CH)]
            WV = [persist.tile([128, F], F16, tag=f"wv{c}", name=f"WV{c}")
                  for c in range(CH)]
            WO = [persist.tile([128, OUTW], F16, tag=f"wo{c}", name=f"WO{c}")
                  for c in range(CH)]
            for c in range(CH):
                sl_c = slice(c * 128, (c + 1) * 128)
                nc.sync.dma_start(out=WQ[c], in_=wq[sl_c, :])
                nc.sync.dma_start(out=WK[c], in_=wk[sl_c, :])
                nc.sync.dma_start(out=WV[c], in_=wv[sl_c, :])
                nc.sync.dma_start(out=WO[c], in_=wo[sl_c, :])

            # ---- persistent attention operands ----
            KT = [persist.tile([128, sl], F16, tag=f"kt{f}", name=f"KT{f}")
                  for f in range(FG)]
            VP = [persist.tile([128, VW], F16, tag=f"vp{k}", name=f"VP{k}")
                  for k in range(NKB)]

            y_local = dram.tile([F, sl], F16)
            y_all = dram.tile([2, F, sl], F16)

            for p in range(NP):
                # ---------- projection for panel p ----------
                xT = [xtp.tile([128, PANEL], F16, tag=f"xt{c}", name=f"xT{c}")
                      for c in range(CH)]
                for sub in range(4):
                    rows = p * PANEL + sub * 128
                    x_t = xload.tile([128, ne], F16, name="x_t")
                    nc.sync.dma_start(out=x_t, in_=x[rows:rows + 128, :])
                    for c in range(CH):
                        ps_t = psp.tile([128, 128], F16, tag="tr", name="ps_t")
                        nc.tensor.transpose(
                            ps_t, x_t[:, c * 128:(c + 1) * 128], ident)
                        nc.vector.tensor_copy(
                            xT[c][:, sub * 128:(sub + 1) * 128], ps_t)

                # Q^T and K^T for this panel: [F, PANEL] in FG tiles
                QT = [qtp.tile([128, PANEL], F16, tag=f"qt{f}", name=f"QT{f}")
                      for f in range(FG)]
                for f in range(FG):
                    for wtiles, dst, bias in (
                        (WQ, QT[f], bqt), (WK, KT[f], bkt)):
                        ps_a = psp.tile([128, PANEL], F32, tag="acc", name="ps_a")
                        for c in range(CH):
                            nc.tensor.matmul(
                                ps_a, wtiles[c][:, f * 128:(f + 1) * 128],
                                xT[c], start=(c == 0), stop=(c == CH - 1))
                        if dst is QT[f]:
                            nc.vector.tensor_scalar_add(
                                dst, ps_a, bias[:, f:f + 1])
                        else:
                            nc.vector.tensor_scalar_add(
                                dst[:, p * PANEL:(p + 1) * PANEL], ps_a,
                                bias[:, f:f + 1])

                # V for this panel -> V' tiles (65-stride, ones col)
                for sub in range(4):
                    kb = p * 4 + sub
                    ps_v = psp.tile([128, F], F32, tag="acc", name="ps_v")
                    for c in range(CH):
                        nc.tensor.matmul(
                            ps_v, xT[c][:, sub * 128:(sub + 1) * 128],
                            WV[c], start=(c == 0), stop=False)
                    # + bias row (broadcast via K=1 matmul)
                    nc.tensor.matmul(ps_v, ones, bvt, start=False, stop=True)
                    vp3 = VP[kb].rearrange("p (h e) -> p h e", e=65)
                    nc.vector.memset(vp3[:, :, 64:65], 1.0)
                    nc.vector.tensor_copy(
                        vp3[:, :, 0:64],
                        ps_v.rearrange("p (h d) -> p h d", d=64))

                # ---------- attention for panel p ----------
                nY = [nyp.tile([64, PANEL], F16, tag=f"ny{h}", name=f"nY{h}")
                      for h in range(H)]
                for h in range(H):
                    f, row = h // 2, (h % 2) * 64
                    ps_y = psp.tile([65, PANEL], F32, tag="y", name="ps_y")
                    nkb_p = 4 * p + 4
                    for kb in range(nkb_p):
                        d = max(0, (kb - 4 * p) * 128)
                        n = PANEL - d
                        ps_s = psp.tile([128, PANEL], F32, tag="s", name="ps_s")
                        nc.tensor.matmul(
                            ps_s[:, 0:n],
                            KT[f][row:row + 64, kb * 128:(kb + 1) * 128],
                            QT[f][row:row + 64, d:PANEL])
                        es = esp.tile([128, PANEL], F16, tag="es", name="es")
                        nc.scalar.activation(
                            es[:, 0:n], ps_s[:, 0:n],
                            mybir.ActivationFunctionType.Exp)
                        if kb >= 4 * p:
                            nc.vector.tensor_mul(
                                es[:, 0:128], es[:, 0:128], tri)
                        nc.tensor.matmul(
                            ps_y[:, d:PANEL],
                            VP[kb][:, h * 65:h * 65 + 65],
                            es[:, 0:n],
                            start=(kb == 0), stop=(kb == nkb_p - 1))
                    # normalize: 1/denom broadcast over the 64 hd rows
                    recip = misc.tile([1, PANEL], F32, tag="recip", name="recip")
                    nc.vector.reciprocal(recip, ps_y[64:65, :])
                    rec16 = misc.tile([1, PANEL], F16, tag="rec16", name="rec16")
                    nc.vector.tensor_copy(rec16, recip)
                    bc = psp.tile([64, PANEL], F32, tag="s", name="bc")
                    nc.tensor.matmul(bc, ones[:, 0:64], rec16)
                    nc.scalar.copy(nY[h], ps_y[0:64, :])
                    nc.vector.tensor_mul(nY[h], nY[h], bc)
                for h in range(H):
                    nc.sync.dma_start(
                        out=y_local[h * 64:(h + 1) * 64,
                                    p * PANEL:(p + 1) * PANEL],
                        in_=nY[h])

            # ---------- exchange y between the two cores of this batch ----
            nc.gpsimd.collective_compute(
                "AllGather",
                mybir.AluOpType.bypass,
                replica_groups=[[0, 1], [2, 3], [4, 5], [6, 7]],
                ins=[y_local.opt()],
                outs=[y_all.opt()],
            )
            y_flat = y_all.rearrange("g f s -> (g f) s")

            # ---------- out projection (our 512-column half) ----------
            for sb in range(sl // 128):
                ps_o = psp.tile([128, OUTW], F32, tag="acc", name="ps_o")
                for c in range(CH):
                    y_t = misc.tile([128, 128], F16, tag="y_t", bufs=4,
                                    name="y_t")
                    nc.sync.dma_start(
                        out=y_t,
                        in_=y_flat[c * 128:(c + 1) * 128,
                                   sb * 128:(sb + 1) * 128])
                    nc.tensor.matmul(ps_o, y_t, WO[c],
                                     start=(c == 0), stop=False)
                nc.tensor.matmul(ps_o, ones, bot, start=False, stop=True)
                o_t = misc.tile([128, OUTW], F32, tag="o_t", bufs=3, name="o_t")
                nc.vector.tensor_copy(o_t, ps_o)
                nc.sync.dma_start(
                    out=out[sb * 128:(sb + 1) * 128, :], in_=o_t)

    nc.compile()
    return nc


def shard_inputs(x, mask, Wqkv, bqkv, Wo, bo, sl=SL, ne=NE, nh=NH):
    """Host-side sharding: returns in_maps for the 8 cores."""
    H = nh // 2
    F = H * HD
    scale = 1.0 / np.sqrt(HD)
    in_maps = []
    for c in range(N_CORES):
        b, g = c // 2, c % 2
        qc = slice(g * F, (g + 1) * F)
        kc = slice(ne + g * F, ne + (g + 1) * F)
        vc = slice(2 * ne + g * F, 2 * ne + (g + 1) * F)
        oc = slice(g * (ne // 2), (g + 1) * (ne // 2))
        in_maps.append({
            "x": np.ascontiguousarray(x[b]).astype(np.float16),
            "wq": (np.ascontiguousarray(Wqkv[:, qc]) * scale).astype(np.float16),
            "wk": np.ascontiguousarray(Wqkv[:, kc]).astype(np.float16),
            "wv": np.ascontiguousarray(Wqkv[:, vc]).astype(np.float16),
            "bq": np.ascontiguousarray(bqkv[qc]) * scale,
            "bk": np.ascontiguousarray(bqkv[kc]),
            "bv": np.ascontiguousarray(bqkv[vc]).astype(np.float16),
            "wo": np.ascontiguousarray(Wo[:, oc]).astype(np.float16),
            "bo": np.ascontiguousarray(bo[oc]).astype(np.float16),
        })
    return in_maps


def unshard_output(results, sl=SL, ne=NE):
    out = np.empty((BS, sl, ne), dtype=np.float32)
    half = ne // 2
    for c in range(N_CORES):
        b, g = c // 2, c % 2
        out[b, :, g * half:(g + 1) * half] = results[c]["out"]
    return out


_NC_CACHE = {}


def kernel(x, mask, Wqkv, bqkv, Wo, bo):
    x = np.asarray(x, dtype=np.float32)
    Wqkv = np.asarray(Wqkv, dtype=np.float32)
    bqkv = np.asarray(bqkv, dtype=np.float32)
    Wo = np.asarray(Wo, dtype=np.float32)
    bo = np.asarray(bo, dtype=np.float32)
    if "nc" not in _NC_CACHE:
        _NC_CACHE["nc"] = build_nc()
    nc = _NC_CACHE["nc"]
    in_maps = shard_inputs(x, mask, Wqkv, bqkv, Wo, bo)
    res = run_bass_kernel_spmd(nc, in_maps, list(range(N_CORES)))
    return unshard_output(res.results)
